# revision 7
# baseline (speedup 1.0000x reference)
"""Trainium2 Bass kernel for nn_MultiHeadAttention_28028956574019.

Sparse windowed multi-head attention, G=4 window groups, learned per-row
window offset. Data-parallel over batch: 8 NeuronCores, one batch element
per core.

Per-core device program (L=2048, H=1024, d=256 per group):
  offset path (folded): host precomputes woffl = off_w.T @ lin2_w.T [H,1];
      device: relu(x) (bf16, from resident qtb) -> tiny matmul -> sigmoid;
      mask row W = BIG*(q_idx + dx) broadcast to [128, 2048] via gpsimd.
  phase 2: Q/K projection (bf16 matmul); PSUM copied to fp8-e4m3 pair
      layout QT8/KT8 [128, 2, 2048] (x16 scale) by the scalar engine.
  phase 3: V projection (bf16): VT [l, 4*(256+1)] with ones columns.
  phase 4: per group, per 512-wide q-strip, k-blocks descending:
      S_T[k,q] = one DoubleRow fp8 matmul (256-contraction in a single
      pass, PSUM = 256*S_raw), masked via
      z = min(S, W - BIG*(k - ws)) (DVE scalar_tensor_tensor) with
      diagonal lower-bound min on GPSIMD; exp on ACT over packed ~1024
      col batches -> bf16 P; out += P.T@V bf16 (ones col gives denom).
      Numerator+denominator DMA'd straight from PSUM; host divides.

All shapes hardcoded for the fixed problem size. The harness calls
kernel(**inputs) with the full (unsharded) inputs.
"""

import sys

if "/opt/trn_rl_repo" not in sys.path:
    sys.path.insert(0, "/opt/trn_rl_repo")

import numpy as np
import ml_dtypes

import concourse.bass as bass  # noqa: F401  (bass must import before bacc)
from concourse import bacc
import concourse.mybir as mybir
from concourse.tile import TileContext
from concourse.bass_utils import run_bass_kernel_spmd

dt = mybir.dt
AF = mybir.ActivationFunctionType
Alu = mybir.AluOpType
PM = mybir.MatmulPerfMode

B, L, H = 8, 2048, 1024
G, D = 4, 256          # groups, per-group head dim
D1 = 256               # learned-offset hidden dim
WS = [4, 16, 64, 256]
BIG = 1.0e7
QS = 16.0              # fp8 scale applied to Q and K
SCALE2E = 2.0 / (float(np.sqrt(L)) * QS * QS)
NCORES = 8

_BUILT = None


def build_nc():
    nc = bacc.Bacc("TRN2", target_bir_lowering=False, debug=False)

    # ---- I/O ----
    qtbf = nc.declare_dram_parameter("qtbf", [H, L], dt.bfloat16, isOutput=False)
    wqk = nc.declare_dram_parameter("wqk", [H, 2 * H], dt.bfloat16, isOutput=False)
    wv = nc.declare_dram_parameter("wv", [H, H], dt.bfloat16, isOutput=False)
    woffl = nc.declare_dram_parameter("woffl", [128, 8], dt.bfloat16, isOutput=False)
    bqk = nc.declare_dram_parameter("bqk", [128, 16], dt.float32, isOutput=False)
    bv = nc.declare_dram_parameter("bv", [128, H], dt.bfloat16, isOutput=False)
    blin2 = nc.declare_dram_parameter("blin2", [1, 1], dt.float32, isOutput=False)
    dtile = nc.declare_dram_parameter("dtile", [128, 128], dt.bfloat16, isOutput=False)
    iotab = nc.declare_dram_parameter("iotab", [1, L], dt.float32, isOutput=False)
    kvec = nc.declare_dram_parameter("kvec", [128, 64], dt.float32, isOutput=False)
    out = nc.declare_dram_parameter("out", [L, H], dt.float32, isOutput=True)

    with TileContext(nc) as tc:
        with tc.tile_pool(name="persist", bufs=1) as pp:
            # ---- consts ----
            bqk_t = pp.tile([128, 16], dt.float32, name="bqk_t")
            nc.gpsimd.dma_start(out=bqk_t[:], in_=bqk[:])
            woffl_t = pp.tile([128, 8], dt.bfloat16, name="woffl_t")
            nc.gpsimd.dma_start(out=woffl_t[:], in_=woffl[:])
            blin2_t = pp.tile([1, 1], dt.float32, name="blin2_t")
            nc.gpsimd.dma_start(out=blin2_t[:], in_=blin2[:])
            dt_t = pp.tile([128, 128], dt.bfloat16, name="dt_t")
            nc.gpsimd.dma_start(out=dt_t[:], in_=dtile[:])
            kvec_t = pp.tile([128, 64], dt.float32, name="kvec_t")
            nc.gpsimd.dma_start(out=kvec_t[:], in_=kvec[:])
            iotab_t = pp.tile([1, L], dt.float32, name="iotab_t")
            nc.gpsimd.dma_start(out=iotab_t[:], in_=iotab[:])
            bvb = pp.tile([128, H], dt.bfloat16, name="bvb")
            nc.gpsimd.dma_start(out=bvb[:], in_=bv[:])

            # bf16 query (transposed), strip-major so early strips land first
            qtb = [pp.tile([128, L], dt.bfloat16, name=f"qtb{i}") for i in range(8)]
            for s in range(4):
                for hin in range(8):
                    nc.gpsimd.dma_start(
                        out=qtb[hin][:, s * 512:(s + 1) * 512],
                        in_=qtbf[hin * 128:(hin + 1) * 128, s * 512:(s + 1) * 512])

            # bf16 V-projection weights (needed only in phase 3)
            wv_t = []
            for i in range(8):
                t = pp.tile([128, H], dt.bfloat16, name=f"wv{i}")
                nc.gpsimd.dma_start(out=t[:], in_=wv[i * 128:(i + 1) * 128, :])
                wv_t.append(t)

            # persistent fp8 Q/K in DoubleRow pair layout [128d, 2half, L]
            QT8 = [pp.tile([128, 2, L], dt.float8e4, name=f"QT8{g}") for g in range(G)]
            KT8 = [pp.tile([128, 2, L], dt.float8e4, name=f"KT8{g}") for g in range(G)]

            # V (natural layout) + ones column per group
            VT = []
            for lb in range(16):
                t = pp.tile([128, 4 * (D + 1)], dt.bfloat16, name=f"VT{lb}",
                            tag=f"VT{lb}")
                nc.vector.memset(t[:, D::D + 1], 1.0)
                VT.append(t)

            sig_row = pp.tile([1, L], dt.float32, name="sig_row")
            wbig = pp.tile([128, L], dt.float32, name="wbig")

            # ============ phase 2: Q/K projection (+ offset path) ============
            with tc.tile_pool(name="p2", bufs=1) as p2, \
                 tc.tile_pool(name="ps2", bufs=3, space="PSUM") as ps2, \
                 tc.tile_pool(name="ps1", bufs=2, space="PSUM") as ps1:
                for hb in range(16):
                    wt = []
                    for hin in range(8):
                        t = p2.tile([128, 128], dt.bfloat16, tag="wqk", bufs=16)
                        nc.sync.dma_start(
                            out=t[:],
                            in_=wqk[hin * 128:(hin + 1) * 128, hb * 128:(hb + 1) * 128])
                        wt.append(t)
                    g, h = (hb % 8) // 2, hb % 2
                    dest = QT8[g] if hb < 8 else KT8[g]
                    for s in range(4):
                        pps = ps2.tile([128, 512], dt.float32, tag="qkps")
                        for hin in range(8):
                            nc.tensor.matmul(pps[:], wt[hin][:],
                                             qtb[hin][:, s * 512:(s + 1) * 512],
                                             start=(hin == 0), stop=(hin == 7))
                        nc.scalar.activation(
                            dest[:, h:h + 1, s * 512:(s + 1) * 512], pps[:],
                            AF.Identity, bias=bqk_t[:, hb:hb + 1], scale=QS)
                    if hb % 4 == 3:
                        sdx = hb // 4
                        zps = ps1.tile([1, 512], dt.float32, tag="zps")
                        for hin in range(8):
                            relu_s = p2.tile([128, 512], dt.bfloat16,
                                             tag="relu", bufs=2, name="relu_s")
                            nc.vector.tensor_relu(
                                out=relu_s[:],
                                in_=qtb[hin][:, sdx * 512:(sdx + 1) * 512])
                            nc.tensor.matmul(zps[:], woffl_t[:, hin:hin + 1],
                                             relu_s[:],
                                             start=(hin == 0), stop=(hin == 7))
                        nc.scalar.activation(sig_row[:, sdx * 512:(sdx + 1) * 512],
                                             zps[:], AF.Sigmoid,
                                             bias=blin2_t[:], scale=1.0)

                nc.vector.scalar_tensor_tensor(sig_row[:], sig_row[:],
                                               float(BIG * L), iotab_t[:],
                                               op0=Alu.mult, op1=Alu.add)
                nc.gpsimd.partition_broadcast(wbig[:], sig_row[:], channels=128)

            # ================= phase 3: V projection =================
            with tc.tile_pool(name="ps3", bufs=3, space="PSUM") as ps3:
                for lb in range(16):
                    for h in range(2):
                        vps = ps3.tile([128, 512], dt.float32, tag="vps")
                        for hin in range(8):
                            nc.tensor.matmul(vps[:],
                                             qtb[hin][:, lb * 128:(lb + 1) * 128],
                                             wv_t[hin][:, h * 512:(h + 1) * 512],
                                             start=(hin == 0), stop=(hin == 7))
                        for gg in range(2):
                            g2 = h * 2 + gg
                            nc.vector.tensor_tensor(
                                out=VT[lb][:, g2 * (D + 1):g2 * (D + 1) + D],
                                in0=vps[:, gg * D:(gg + 1) * D],
                                in1=bvb[:, g2 * D:(g2 + 1) * D], op=Alu.add)

            # ================= phase 4: attention =================
            with tc.tile_pool(name="p4", bufs=1) as p4, \
                 tc.tile_pool(name="pss", bufs=3, space="PSUM") as pss, \
                 tc.tile_pool(name="pso", bufs=5, space="PSUM") as pso:
                for g in range(G):
                    for s in range(4):
                        q0 = s * 512
                        outps = [pso.tile([128, 512], dt.float32, tag="outps",
                                          name="outps") for _ in range(4)]

                        def consume(kb, pt, off):
                            for j in range(4):
                                if 4 * s + j <= kb:
                                    nc.tensor.matmul(
                                        outps[j][:, :D + 1],
                                        pt[:, off + j * 128:off + (j + 1) * 128],
                                        VT[kb][:, g * (D + 1):(g + 1) * (D + 1)],
                                        start=(kb == 15), stop=(kb == 4 * s + j))

                        def width(kb):
                            return 512 if kb >= 4 * s + 3 else (kb - 4 * s + 1) * 128

                        pending = []
                        z2 = None
                        zoff = 0
                        zrec = []
                        for kb in range(15, 4 * s - 1, -1):
                            w = width(kb)
                            sps = pss.tile([128, 512], dt.float32, tag="sps")
                            nc.tensor.matmul(sps[:, :w],
                                             KT8[g][:, :, kb * 128:(kb + 1) * 128],
                                             QT8[g][:, :, q0:q0 + w],
                                             start=True, stop=True,
                                             perf_mode=PM.DoubleRow)
                            if len(pending) >= 4:
                                consume(*pending.pop(0))
                            if z2 is None:
                                z2 = p4.tile([128, 1024], dt.bfloat16, tag="z",
                                             bufs=4, name="z2")
                                zoff, zrec = 0, []
                            nc.vector.scalar_tensor_tensor(
                                z2[:, zoff:zoff + w], wbig[:, q0:q0 + w],
                                kvec_t[:, g * 16 + kb:g * 16 + kb + 1], sps[:, :w],
                                op0=Alu.subtract, op1=Alu.min)
                            if kb <= 4 * s + 3:
                                nc.vector.tensor_tensor(
                                    out=z2[:, zoff + w - 128:zoff + w],
                                    in0=z2[:, zoff + w - 128:zoff + w],
                                    in1=dt_t[:], op=Alu.min)
                            zrec.append((kb, zoff))
                            zoff += w
                            if kb == 4 * s or zoff + width(kb - 1) > 1024:
                                pt2 = p4.tile([128, 1024], dt.bfloat16, tag="pt",
                                              bufs=4, name="pt2")
                                nc.scalar.activation(pt2[:, :zoff], z2[:, :zoff],
                                                     AF.Exp, scale=SCALE2E)
                                for kbx, offx in zrec:
                                    pending.append((kbx, pt2, offx))
                                z2 = None
                        for it in pending:
                            consume(*it)
                        for j in (3, 2, 1, 0):
                            c = s * 4 + j
                            rden = p4.tile([128, 1], dt.float32, tag="rden",
                                           bufs=4, name="rden")
                            nc.vector.reciprocal(out=rden[:],
                                                 in_=outps[j][:, D:D + 1])
                            outn = p4.tile([128, D], dt.float32, tag="outn",
                                           bufs=4, name="outn")
                            nc.scalar.mul(outn[:], outps[j][:, :D], rden[:])
                            nc.sync.dma_start(
                                out=out[c * 128:(c + 1) * 128, g * D:(g + 1) * D],
                                in_=outn[:])

    nc.finalize()
    return nc


def _prep_shared(qkv_w, qkv_b, off_w, lin2_w, lin2_b):
    f32 = np.float32
    bf = ml_dtypes.bfloat16
    qkv_wT = np.ascontiguousarray(qkv_w.T, dtype=f32)          # [H, 3H]
    woffl = (off_w.T.astype(np.float64) @ lin2_w.T.astype(np.float64))  # [H, 1]
    shared = {
        "wqk": qkv_wT[:, :2 * H].astype(bf),
        "wv": np.ascontiguousarray(qkv_wT[:, 2 * H:]).astype(bf),
        "woffl": np.ascontiguousarray(woffl.reshape(8, 128).T).astype(bf),
        "bqk": np.ascontiguousarray(
            QS * qkv_b[:2 * H].reshape(16, 128).T, dtype=f32),
        "bv": np.ascontiguousarray(
            np.broadcast_to(qkv_b[2 * H:][None], (128, H))).astype(bf),
        "blin2": np.asarray(lin2_b, dtype=f32).reshape(1, 1),
        "iotab": (BIG * np.arange(L, dtype=np.float64)).astype(f32)[None],
    }
    p = np.arange(128, dtype=np.float64)[:, None]
    cols = []
    for g in range(G):
        for kb in range(16):
            cols.append(BIG * (kb * 128 + p - WS[g]))
    shared["kvec"] = np.concatenate(cols, axis=1).astype(f32)
    pi = np.arange(128)[:, None]
    fi = np.arange(128)[None, :]
    shared["dtile"] = np.where(pi >= fi, 1e6, -1e6).astype(bf)
    return shared


def kernel(query, key_in, value, qkv_w, qkv_b, off_w, lin2_w, lin2_b,
           _trace=False, _tmpdir=None):
    global _BUILT
    query = np.asarray(query, dtype=np.float32)
    shared = _prep_shared(np.asarray(qkv_w, np.float32),
                          np.asarray(qkv_b, np.float32),
                          np.asarray(off_w, np.float32),
                          np.asarray(lin2_w, np.float32),
                          np.asarray(lin2_b, np.float32))
    in_maps = []
    for b in range(NCORES):
        m = dict(shared)
        m["qtbf"] = np.ascontiguousarray(query[b].T).astype(ml_dtypes.bfloat16)
        in_maps.append(m)

    if _BUILT is None:
        _BUILT = build_nc()
    kw = {}
    if _trace:
        kw = dict(trace=True, tmpdir=_tmpdir)
    res = run_bass_kernel_spmd(_BUILT, in_maps, core_ids=list(range(NCORES)), **kw)
    out = np.stack([np.asarray(res.results[b]["out"]) for b in range(NCORES)],
                   axis=0)
    if _trace:
        return out, res
    return out


if __name__ == "__main__":
    rng = np.random.default_rng(0)
    ins = {
        "query": rng.standard_normal((B, L, H)).astype(np.float32),
        "key_in": rng.standard_normal((B, L, H)).astype(np.float32),
        "value": rng.standard_normal((B, L, H)).astype(np.float32),
        "qkv_w": (rng.standard_normal((3 * H, H)) * 0.02).astype(np.float32),
        "qkv_b": np.zeros(3 * H, np.float32),
        "off_w": (rng.standard_normal((D1, H)) * 0.02).astype(np.float32),
        "lin2_w": (rng.standard_normal((1, D1)) * 0.02).astype(np.float32),
        "lin2_b": np.zeros(1, np.float32),
    }
    o = kernel(**ins)
    print("out", o.shape, o.dtype, np.abs(o).mean())


# revision 13
# speedup vs baseline: 1.1403x; 1.1403x over previous
"""Trainium2 Bass kernel for nn_MultiHeadAttention_28028956574019.

Sparse windowed multi-head attention, G=4 window groups, learned per-row
window offset. Data-parallel over batch: 8 NeuronCores, one batch element
per core.

Per-core device program (L=2048, H=1024, d=256 per group):
  offset path (folded): host precomputes woffl = off_w.T @ lin2_w.T [H,1];
      device: relu(x) (bf16, from resident qtb) -> tiny matmul -> sigmoid;
      mask row W = BIG*(q_idx + dx) broadcast to [128, 2048] via gpsimd.
  phase 2: Q/K projection (bf16); PSUM copied to bf16 QT/KT (ACT, +bias).
  phase 3: V projection (bf16): VT [l, 4*(256+1)] with ones columns.
  phase 4: per group, per 512-wide q-strip, k-blocks descending:
      S_T[k,q] = K_T.T@Q_T (2 bf16 matmuls, PSUM f32). The host computes
      dx exactly (it only depends on query+weights) and derives a block
      plan: fully-masked blocks are skipped outright; fully-inside blocks
      skip the mask and exp straight from PSUM; boundary blocks use
      z = min(S, W - BIG*(k - ws)) (DVE) with a diagonal lower-bound min,
      packed into ~1024-col z2 tiles so one ACT exp covers two blocks.
      out += p.T@V (ones col gives denominator); epilogue = reciprocal
      (DVE) + per-partition-scaled copy (ACT) + DMA.

The compiled program is cached keyed by the block plan; for a fixed input
distribution it compiles once.
"""

import sys

if "/opt/trn_rl_repo" not in sys.path:
    sys.path.insert(0, "/opt/trn_rl_repo")

import numpy as np
import ml_dtypes

import concourse.bass as bass  # noqa: F401  (bass must import before bacc)
from concourse import bacc
import concourse.mybir as mybir
from concourse.tile import TileContext
from concourse.bass_utils import run_bass_kernel_spmd

dt = mybir.dt
AF = mybir.ActivationFunctionType
Alu = mybir.AluOpType

B, L, H = 8, 2048, 1024
G, D = 4, 256          # groups, per-group head dim
D1 = 256               # learned-offset hidden dim
WS = [4, 16, 64, 256]
BIG = 1.0e7
SCALE2 = 2.0 / float(np.sqrt(L))   # masked_fill+add doubles unmasked scores
MARGIN = 16            # safety margin (keys) for host-side block decisions
NCORES = 8

_CACHE = {}


def build_nc(plan):
    """plan: dict (g, s) -> (amax, nmax): blocks kb in [amax..4s] are live,
    blocks in [4s+4..nmax] need no mask."""
    nc = bacc.Bacc("TRN2", target_bir_lowering=False, debug=False)

    # ---- I/O ----  (host pre-permutes to partition-major 3D layouts so each
    # logical load is ONE dma descriptor instead of eight)
    qtbf = nc.declare_dram_parameter("qtbf", [128, 8, L], dt.bfloat16,
                                     isOutput=False)
    wqk = nc.declare_dram_parameter("wqk", [128, 16, 8, 128], dt.bfloat16,
                                    isOutput=False)
    wv = nc.declare_dram_parameter("wv", [128, 8, H], dt.bfloat16, isOutput=False)
    woffl = nc.declare_dram_parameter("woffl", [128, 8], dt.bfloat16, isOutput=False)
    bqk = nc.declare_dram_parameter("bqk", [128, 16], dt.float32, isOutput=False)
    bv = nc.declare_dram_parameter("bv", [128, H], dt.bfloat16, isOutput=False)
    blin2 = nc.declare_dram_parameter("blin2", [1, 1], dt.float32, isOutput=False)
    dtile = nc.declare_dram_parameter("dtile", [128, 128], dt.bfloat16, isOutput=False)
    iotab = nc.declare_dram_parameter("iotab", [1, L], dt.float32, isOutput=False)
    kvec = nc.declare_dram_parameter("kvec", [128, 64], dt.float32, isOutput=False)
    out = nc.declare_dram_parameter("out", [L, H], dt.float32, isOutput=True)

    with TileContext(nc) as tc:
        with tc.tile_pool(name="persist", bufs=1) as pp:
            # ---- consts ----
            bqk_t = pp.tile([128, 16], dt.float32, name="bqk_t")
            nc.gpsimd.dma_start(out=bqk_t[:], in_=bqk[:])
            woffl_t = pp.tile([128, 8], dt.bfloat16, name="woffl_t")
            nc.gpsimd.dma_start(out=woffl_t[:], in_=woffl[:])
            blin2_t = pp.tile([1, 1], dt.float32, name="blin2_t")
            nc.gpsimd.dma_start(out=blin2_t[:], in_=blin2[:])
            dt_t = pp.tile([128, 128], dt.bfloat16, name="dt_t")
            nc.gpsimd.dma_start(out=dt_t[:], in_=dtile[:])
            kvec_t = pp.tile([128, 64], dt.float32, name="kvec_t")
            nc.gpsimd.dma_start(out=kvec_t[:], in_=kvec[:])
            iotab_t = pp.tile([1, L], dt.float32, name="iotab_t")
            nc.gpsimd.dma_start(out=iotab_t[:], in_=iotab[:])
            bvb = pp.tile([128, H], dt.bfloat16, name="bvb")
            nc.gpsimd.dma_start(out=bvb[:], in_=bv[:])

            # bf16 query (transposed), strip-major so early strips land first;
            # one dma per 512-col strip covers all 8 h-blocks
            qtball = pp.tile([128, 8, L], dt.bfloat16, name="qtball")
            for s in range(4):
                nc.gpsimd.dma_start(
                    out=qtball[:, :, s * 512:(s + 1) * 512],
                    in_=qtbf[:, :, s * 512:(s + 1) * 512])
            qtb = [qtball[:, i, :] for i in range(8)]

            # bf16 V-projection weights (needed only in phase 3), one dma
            wvall = pp.tile([128, 8, H], dt.bfloat16, name="wvall")
            nc.gpsimd.dma_start(out=wvall[:], in_=wv[:])
            wv_t = [wvall[:, i, :] for i in range(8)]

            # persistent bf16 Q_T / K_T (two d-halves per group)
            QT = [[pp.tile([128, L], dt.bfloat16, name=f"QT{g}{h}") for h in range(2)]
                  for g in range(G)]
            KT = [[pp.tile([128, L], dt.bfloat16, name=f"KT{g}{h}") for h in range(2)]
                  for g in range(G)]

            # V (natural layout) + ones column per group
            VT = []
            for lb in range(16):
                t = pp.tile([128, 4 * (D + 1)], dt.bfloat16, name=f"VT{lb}",
                            tag=f"VT{lb}")
                nc.vector.memset(t[:, D::D + 1], 1.0)
                VT.append(t)

            sig_row = pp.tile([1, L], dt.float32, name="sig_row")
            wbig = pp.tile([128, L], dt.float32, name="wbig")

            # ============ phase 2: Q/K projection (+ offset path) ============
            with tc.tile_pool(name="p2", bufs=1) as p2, \
                 tc.tile_pool(name="ps2", bufs=3, space="PSUM") as ps2, \
                 tc.tile_pool(name="ps1", bufs=2, space="PSUM") as ps1:
                for hb in range(16):
                    wtall = p2.tile([128, 8, 128], dt.bfloat16, tag="wqk", bufs=4)
                    nc.sync.dma_start(out=wtall[:], in_=wqk[:, hb, :, :])
                    wt = [wtall[:, i, :] for i in range(8)]
                    g, h = (hb % 8) // 2, hb % 2
                    dest = QT[g][h] if hb < 8 else KT[g][h]
                    for s in range(4):
                        pps = ps2.tile([128, 512], dt.float32, tag="qkps")
                        for hin in range(8):
                            nc.tensor.matmul(pps[:], wt[hin][:],
                                             qtb[hin][:, s * 512:(s + 1) * 512],
                                             start=(hin == 0), stop=(hin == 7))
                        nc.scalar.activation(dest[:, s * 512:(s + 1) * 512], pps[:],
                                             AF.Identity, bias=bqk_t[:, hb:hb + 1],
                                             scale=1.0)
                    if hb % 4 == 3:
                        sdx = hb // 4
                        zps = ps1.tile([1, 512], dt.float32, tag="zps")
                        for hin in range(8):
                            relu_s = p2.tile([128, 512], dt.bfloat16,
                                             tag="relu", bufs=2, name="relu_s")
                            nc.vector.tensor_relu(
                                out=relu_s[:],
                                in_=qtb[hin][:, sdx * 512:(sdx + 1) * 512])
                            nc.tensor.matmul(zps[:], woffl_t[:, hin:hin + 1],
                                             relu_s[:],
                                             start=(hin == 0), stop=(hin == 7))
                        nc.scalar.activation(sig_row[:, sdx * 512:(sdx + 1) * 512],
                                             zps[:], AF.Sigmoid,
                                             bias=blin2_t[:], scale=1.0)

                nc.vector.scalar_tensor_tensor(sig_row[:], sig_row[:],
                                               float(BIG * L), iotab_t[:],
                                               op0=Alu.mult, op1=Alu.add)
                nc.gpsimd.partition_broadcast(wbig[:], sig_row[:], channels=128)

            # ================= phase 3: V projection =================
            with tc.tile_pool(name="ps3", bufs=3, space="PSUM") as ps3:
                for lb in range(16):
                    for h in range(2):
                        vps = ps3.tile([128, 512], dt.float32, tag="vps")
                        for hin in range(8):
                            nc.tensor.matmul(vps[:],
                                             qtb[hin][:, lb * 128:(lb + 1) * 128],
                                             wv_t[hin][:, h * 512:(h + 1) * 512],
                                             start=(hin == 0), stop=(hin == 7))
                        for gg in range(2):
                            g2 = h * 2 + gg
                            nc.vector.tensor_tensor(
                                out=VT[lb][:, g2 * (D + 1):g2 * (D + 1) + D],
                                in0=vps[:, gg * D:(gg + 1) * D],
                                in1=bvb[:, g2 * D:(g2 + 1) * D], op=Alu.add)

            # ================= phase 4: attention =================
            with tc.tile_pool(name="p4", bufs=1) as p4, \
                 tc.tile_pool(name="pss", bufs=3, space="PSUM") as pss, \
                 tc.tile_pool(name="pso", bufs=5, space="PSUM") as pso:
                for g in range(G):
                    for s in range(4):
                        q0 = s * 512
                        amax, nmax = plan[(g, s)]
                        outps = [pso.tile([128, 512], dt.float32, tag="outps",
                                          name="outps") for _ in range(4)]

                        def consume(kb, pt, off):
                            for j in range(4):
                                if 4 * s + j <= kb:
                                    nc.tensor.matmul(
                                        outps[j][:, :D + 1],
                                        pt[:, off + j * 128:off + (j + 1) * 128],
                                        VT[kb][:, g * (D + 1):(g + 1) * (D + 1)],
                                        start=(kb == amax), stop=(kb == 4 * s + j))

                        def width(kb):
                            return 512 if kb >= 4 * s + 3 else (kb - 4 * s + 1) * 128

                        pending = []
                        z2 = None
                        zoff = 0
                        zrec = []

                        def flush_pack():
                            nonlocal z2
                            pt2 = p4.tile([128, 1024], dt.bfloat16, tag="pt",
                                          bufs=4, name="pt2")
                            nc.scalar.activation(pt2[:, :zoff], z2[:, :zoff],
                                                 AF.Exp, scale=SCALE2)
                            for kbx, offx in zrec:
                                pending.append((kbx, pt2, offx))
                            z2 = None

                        for kb in range(amax, 4 * s - 1, -1):
                            w = width(kb)
                            sps = pss.tile([128, 512], dt.float32, tag="sps")
                            nc.tensor.matmul(sps[:, :w],
                                             KT[g][0][:, kb * 128:(kb + 1) * 128],
                                             QT[g][0][:, q0:q0 + w],
                                             start=True, stop=False)
                            nc.tensor.matmul(sps[:, :w],
                                             KT[g][1][:, kb * 128:(kb + 1) * 128],
                                             QT[g][1][:, q0:q0 + w],
                                             start=False, stop=True)
                            if len(pending) >= 4:
                                consume(*pending.pop(0))
                            if 4 * s + 4 <= kb <= nmax:
                                # fully inside the window: no mask needed
                                if z2 is not None:
                                    flush_pack()
                                pt1 = p4.tile([128, 512], dt.bfloat16, tag="pt1",
                                              bufs=4, name="pt1")
                                nc.scalar.activation(pt1[:, :w], sps[:, :w],
                                                     AF.Exp, scale=SCALE2)
                                pending.append((kb, pt1, 0))
                                continue
                            if z2 is None:
                                z2 = p4.tile([128, 1024], dt.bfloat16, tag="z",
                                             bufs=4, name="z2")
                                zoff, zrec = 0, []
                            nc.vector.scalar_tensor_tensor(
                                z2[:, zoff:zoff + w], wbig[:, q0:q0 + w],
                                kvec_t[:, g * 16 + kb:g * 16 + kb + 1], sps[:, :w],
                                op0=Alu.subtract, op1=Alu.min)
                            if kb <= 4 * s + 3:
                                nc.vector.tensor_tensor(
                                    out=z2[:, zoff + w - 128:zoff + w],
                                    in0=z2[:, zoff + w - 128:zoff + w],
                                    in1=dt_t[:], op=Alu.min)
                            zrec.append((kb, zoff))
                            zoff += w
                            if kb == 4 * s or zoff + width(kb - 1) > 1024:
                                flush_pack()
                        if z2 is not None:
                            flush_pack()
                        for it in pending:
                            consume(*it)
                        for j in (3, 2, 1, 0):
                            c = s * 4 + j
                            rden = p4.tile([128, 1], dt.float32, tag="rden",
                                           bufs=4, name="rden")
                            nc.vector.reciprocal(out=rden[:],
                                                 in_=outps[j][:, D:D + 1])
                            outn = p4.tile([128, D], dt.float32, tag="outn",
                                           bufs=4, name="outn")
                            nc.scalar.mul(outn[:], outps[j][:, :D], rden[:])
                            nc.sync.dma_start(
                                out=out[c * 128:(c + 1) * 128, g * D:(g + 1) * D],
                                in_=outn[:])

    nc.finalize()
    return nc


def _make_plan(query, woffl_np, lin2_b):
    """Host-exact window offsets -> per-(g,s) block plan (batch-uniform)."""
    z = np.maximum(query.astype(np.float64), 0.0).reshape(-1, H) @ woffl_np
    dx = (1.0 / (1.0 + np.exp(-(z + float(lin2_b[0]))))).reshape(B, L) * L
    plan = {}
    q_idx = np.arange(L, dtype=np.float64)
    for g, ws in enumerate(WS):
        lim = q_idx[None, :] + dx + ws          # [B, L] max allowed k (float)
        for s in range(4):
            sl = lim[:, s * 512:(s + 1) * 512]
            amax = 4 * s
            for kb in range(15, 4 * s - 1, -1):
                if not (kb * 128 > sl + MARGIN).all():
                    amax = kb
                    break
            nmax = 4 * s + 3
            for kb in range(min(amax, 15), 4 * s + 3, -1):
                if (kb * 128 + 127 <= sl - MARGIN).all():
                    nmax = kb
                    break
            plan[(g, s)] = (amax, nmax)
    return plan


def _prep_shared(qkv_w, qkv_b, off_w, lin2_w, lin2_b):
    f32 = np.float32
    bf = ml_dtypes.bfloat16
    qkv_wT = np.ascontiguousarray(qkv_w.T, dtype=f32)          # [H, 3H]
    woffl = (off_w.T.astype(np.float64) @ lin2_w.T.astype(np.float64))  # [H, 1]
    # [H, 2H] -> [p, hb, hin, c]; [H, H] -> [p, hin, c]  (partition-major)
    wqk_np = (qkv_wT[:, :2 * H].reshape(8, 128, 16, 128)
              .transpose(1, 2, 0, 3))
    wv_np = qkv_wT[:, 2 * H:].reshape(8, 128, H).transpose(1, 0, 2)
    shared = {
        "wqk": np.ascontiguousarray(wqk_np).astype(bf),
        "wv": np.ascontiguousarray(wv_np).astype(bf),
        "woffl": np.ascontiguousarray(woffl.reshape(8, 128).T).astype(bf),
        "bqk": np.ascontiguousarray(
            qkv_b[:2 * H].reshape(16, 128).T, dtype=f32),
        "bv": np.ascontiguousarray(
            np.broadcast_to(qkv_b[2 * H:][None], (128, H))).astype(bf),
        "blin2": np.asarray(lin2_b, dtype=f32).reshape(1, 1),
        "iotab": (BIG * np.arange(L, dtype=np.float64)).astype(f32)[None],
    }
    p = np.arange(128, dtype=np.float64)[:, None]
    cols = []
    for g in range(G):
        for kb in range(16):
            cols.append(BIG * (kb * 128 + p - WS[g]))
    shared["kvec"] = np.concatenate(cols, axis=1).astype(f32)
    pi = np.arange(128)[:, None]
    fi = np.arange(128)[None, :]
    shared["dtile"] = np.where(pi >= fi, 1e6, -1e6).astype(bf)
    return shared, woffl


def kernel(query, key_in, value, qkv_w, qkv_b, off_w, lin2_w, lin2_b,
           _trace=False, _tmpdir=None):
    query = np.asarray(query, dtype=np.float32)
    shared, woffl_np = _prep_shared(np.asarray(qkv_w, np.float32),
                                    np.asarray(qkv_b, np.float32),
                                    np.asarray(off_w, np.float32),
                                    np.asarray(lin2_w, np.float32),
                                    np.asarray(lin2_b, np.float32))
    plan = _make_plan(query, woffl_np, np.asarray(lin2_b, np.float64).ravel())
    in_maps = []
    for b in range(NCORES):
        m = dict(shared)
        qT = query[b].T.reshape(8, 128, L).transpose(1, 0, 2)  # [p, hin, col]
        m["qtbf"] = np.ascontiguousarray(qT).astype(ml_dtypes.bfloat16)
        in_maps.append(m)

    key = tuple(sorted(plan.items()))
    if key not in _CACHE:
        _CACHE[key] = build_nc(plan)
    kw = {}
    if _trace:
        kw = dict(trace=True, tmpdir=_tmpdir)
    res = run_bass_kernel_spmd(_CACHE[key], in_maps,
                               core_ids=list(range(NCORES)), **kw)
    out = np.stack([np.asarray(res.results[b]["out"]) for b in range(NCORES)],
                   axis=0)
    if _trace:
        return out, res
    return out


if __name__ == "__main__":
    rng = np.random.default_rng(0)
    ins = {
        "query": rng.standard_normal((B, L, H)).astype(np.float32),
        "key_in": rng.standard_normal((B, L, H)).astype(np.float32),
        "value": rng.standard_normal((B, L, H)).astype(np.float32),
        "qkv_w": (rng.standard_normal((3 * H, H)) * 0.02).astype(np.float32),
        "qkv_b": np.zeros(3 * H, np.float32),
        "off_w": (rng.standard_normal((D1, H)) * 0.02).astype(np.float32),
        "lin2_w": (rng.standard_normal((1, D1)) * 0.02).astype(np.float32),
        "lin2_b": np.zeros(1, np.float32),
    }
    o = kernel(**ins)
    print("out", o.shape, o.dtype, np.abs(o).mean())


# revision 18
# speedup vs baseline: 1.1506x; 1.0091x over previous
"""Trainium2 Bass kernel for nn_MultiHeadAttention_28028956574019.

Sparse windowed multi-head attention, G=4 window groups, learned per-row
window offset. Data-parallel over batch: 8 NeuronCores, one batch element
per core.

Per-core device program (L=2048, H=1024, d=256 per group):
  offset path (folded): host precomputes woffl = off_w.T @ lin2_w.T [H,1];
      device: relu(x) (bf16, from resident qtb) -> tiny matmul -> sigmoid;
      mask row W = BIG*(q_idx + dx) broadcast to [128, 2048] via gpsimd.
  phase 2: Q/K projection (bf16); PSUM copied to bf16 QT/KT (ACT, +bias).
  phase 3: V projection (bf16): VT [l, 4*(256+1)] with ones columns.
  phase 4: per group, per 512-wide q-strip, k-blocks descending:
      S_T[k,q] = K_T.T@Q_T (2 bf16 matmuls, PSUM f32). The host computes
      dx exactly (it only depends on query+weights) and derives a block
      plan: fully-masked blocks are skipped outright; fully-inside blocks
      skip the mask and exp straight from PSUM; boundary blocks use
      z = min(S, W - BIG*(k - ws)) (DVE) with a diagonal lower-bound min,
      packed into ~1024-col z2 tiles so one ACT exp covers two blocks.
      out += p.T@V (ones col gives denominator); epilogue = reciprocal
      (DVE) + per-partition-scaled copy (ACT) + DMA.

The compiled program is cached keyed by the block plan; for a fixed input
distribution it compiles once.
"""

import sys

if "/opt/trn_rl_repo" not in sys.path:
    sys.path.insert(0, "/opt/trn_rl_repo")

import numpy as np
import ml_dtypes

import concourse.bass as bass  # noqa: F401  (bass must import before bacc)
from concourse import bacc
import concourse.mybir as mybir
from concourse.tile import TileContext
from concourse.bass_utils import run_bass_kernel_spmd

dt = mybir.dt
AF = mybir.ActivationFunctionType
Alu = mybir.AluOpType

B, L, H = 8, 2048, 1024
G, D = 4, 256          # groups, per-group head dim
D1 = 256               # learned-offset hidden dim
WS = [4, 16, 64, 256]
BIG = 1.0e7
SCALE2 = 2.0 / float(np.sqrt(L))   # masked_fill+add doubles unmasked scores
MARGIN = 16            # safety margin (keys) for host-side block decisions
NCORES = 8

_CACHE = {}


def build_nc(plan):
    """plan: dict (g, s) -> (amax, nmax): blocks kb in [amax..4s] are live,
    blocks in [4s+4..nmax] need no mask."""
    nc = bacc.Bacc("TRN2", target_bir_lowering=False, debug=False)

    # ---- I/O ----  (host pre-permutes to partition-major 3D layouts so each
    # logical load is ONE dma descriptor instead of eight)
    qtbf = nc.declare_dram_parameter("qtbf", [128, 4, 8, 512], dt.bfloat16,
                                     isOutput=False)
    wqk = nc.declare_dram_parameter("wqk", [128, 16, 8, 128], dt.bfloat16,
                                    isOutput=False)
    wv = nc.declare_dram_parameter("wv", [128, 8, H], dt.bfloat16, isOutput=False)
    woffl = nc.declare_dram_parameter("woffl", [128, 8], dt.bfloat16, isOutput=False)
    bqk = nc.declare_dram_parameter("bqk", [128, 16], dt.float32, isOutput=False)
    bv = nc.declare_dram_parameter("bv", [128, H], dt.bfloat16, isOutput=False)
    blin2 = nc.declare_dram_parameter("blin2", [1, 1], dt.float32, isOutput=False)
    dtile = nc.declare_dram_parameter("dtile", [128, 128], dt.bfloat16, isOutput=False)
    iotab = nc.declare_dram_parameter("iotab", [1, L], dt.float32, isOutput=False)
    kvec = nc.declare_dram_parameter("kvec", [128, 64], dt.float32, isOutput=False)
    out = nc.declare_dram_parameter("out", [L, H], dt.float32, isOutput=True)

    with TileContext(nc) as tc:
        with tc.tile_pool(name="persist", bufs=1) as pp:
            # ---- consts ----
            bqk_t = pp.tile([128, 16], dt.float32, name="bqk_t")
            nc.gpsimd.dma_start(out=bqk_t[:], in_=bqk[:])
            woffl_t = pp.tile([128, 8], dt.bfloat16, name="woffl_t")
            nc.gpsimd.dma_start(out=woffl_t[:], in_=woffl[:])
            blin2_t = pp.tile([1, 1], dt.float32, name="blin2_t")
            nc.gpsimd.dma_start(out=blin2_t[:], in_=blin2[:])
            dt_t = pp.tile([128, 128], dt.bfloat16, name="dt_t")
            nc.gpsimd.dma_start(out=dt_t[:], in_=dtile[:])
            kvec_t = pp.tile([128, 64], dt.float32, name="kvec_t")
            nc.gpsimd.dma_start(out=kvec_t[:], in_=kvec[:])
            iotab_t = pp.tile([1, L], dt.float32, name="iotab_t")
            nc.gpsimd.dma_start(out=iotab_t[:], in_=iotab[:])
            bvb = pp.tile([128, H], dt.bfloat16, name="bvb")
            nc.gpsimd.dma_start(out=bvb[:], in_=bv[:])

            # bf16 query (transposed), strip-major so early strips land first;
            # one dma per 512-col strip covers all 8 h-blocks
            qtball = pp.tile([128, 8, L], dt.bfloat16, name="qtball")
            for s in range(4):
                nc.gpsimd.dma_start(
                    out=qtball[:, :, s * 512:(s + 1) * 512],
                    in_=qtbf[:, s, :, :])
            qtb = [qtball[:, i, :] for i in range(8)]

            # bf16 V-projection weights (needed only in phase 3), one dma
            wvall = pp.tile([128, 8, H], dt.bfloat16, name="wvall")
            nc.gpsimd.dma_start(out=wvall[:], in_=wv[:])
            wv_t = [wvall[:, i, :] for i in range(8)]

            # persistent bf16 Q_T / K_T (two d-halves per group)
            QT = [[pp.tile([128, L], dt.bfloat16, name=f"QT{g}{h}") for h in range(2)]
                  for g in range(G)]
            KT = [[pp.tile([128, L], dt.bfloat16, name=f"KT{g}{h}") for h in range(2)]
                  for g in range(G)]

            # V (natural layout) + ones column per group
            VT = []
            for lb in range(16):
                t = pp.tile([128, 4 * (D + 1)], dt.bfloat16, name=f"VT{lb}",
                            tag=f"VT{lb}")
                nc.vector.memset(t[:, D::D + 1], 1.0)
                VT.append(t)

            sig_row = pp.tile([1, L], dt.float32, name="sig_row")
            wbig = pp.tile([128, L], dt.float32, name="wbig")

            # ============ phase 2: Q/K projection (+ offset path) ============
            with tc.tile_pool(name="p2", bufs=1) as p2, \
                 tc.tile_pool(name="ps2", bufs=3, space="PSUM") as ps2, \
                 tc.tile_pool(name="ps1", bufs=2, space="PSUM") as ps1:
                for hb in range(16):
                    wtall = p2.tile([128, 8, 128], dt.bfloat16, tag="wqk", bufs=4)
                    nc.sync.dma_start(out=wtall[:], in_=wqk[:, hb, :, :])
                    wt = [wtall[:, i, :] for i in range(8)]
                    g, h = (hb % 8) // 2, hb % 2
                    dest = QT[g][h] if hb < 8 else KT[g][h]
                    for s in range(4):
                        pps = ps2.tile([128, 512], dt.float32, tag="qkps")
                        for hin in range(8):
                            nc.tensor.matmul(pps[:], wt[hin][:],
                                             qtb[hin][:, s * 512:(s + 1) * 512],
                                             start=(hin == 0), stop=(hin == 7))
                        nc.scalar.activation(dest[:, s * 512:(s + 1) * 512], pps[:],
                                             AF.Identity, bias=bqk_t[:, hb:hb + 1],
                                             scale=1.0)
                    if hb % 4 == 3:
                        sdx = hb // 4
                        zps = ps1.tile([1, 512], dt.float32, tag="zps")
                        for hin in range(8):
                            relu_s = p2.tile([128, 512], dt.bfloat16,
                                             tag="relu", bufs=2, name="relu_s")
                            nc.vector.tensor_relu(
                                out=relu_s[:],
                                in_=qtb[hin][:, sdx * 512:(sdx + 1) * 512])
                            nc.tensor.matmul(zps[:], woffl_t[:, hin:hin + 1],
                                             relu_s[:],
                                             start=(hin == 0), stop=(hin == 7))
                        nc.scalar.activation(sig_row[:, sdx * 512:(sdx + 1) * 512],
                                             zps[:], AF.Sigmoid,
                                             bias=blin2_t[:], scale=1.0)

                nc.vector.scalar_tensor_tensor(sig_row[:], sig_row[:],
                                               float(BIG * L), iotab_t[:],
                                               op0=Alu.mult, op1=Alu.add)
                nc.gpsimd.partition_broadcast(wbig[:], sig_row[:], channels=128)

            # ================= phase 3: V projection =================
            with tc.tile_pool(name="ps3", bufs=3, space="PSUM") as ps3:
                for lb in range(16):
                    for h in range(2):
                        vps = ps3.tile([128, 512], dt.float32, tag="vps")
                        for hin in range(8):
                            nc.tensor.matmul(vps[:],
                                             qtb[hin][:, lb * 128:(lb + 1) * 128],
                                             wv_t[hin][:, h * 512:(h + 1) * 512],
                                             start=(hin == 0), stop=(hin == 7))
                        for gg in range(2):
                            g2 = h * 2 + gg
                            nc.vector.tensor_tensor(
                                out=VT[lb][:, g2 * (D + 1):g2 * (D + 1) + D],
                                in0=vps[:, gg * D:(gg + 1) * D],
                                in1=bvb[:, g2 * D:(g2 + 1) * D], op=Alu.add)

            # ================= phase 4: attention =================
            with tc.tile_pool(name="p4", bufs=1) as p4, \
                 tc.tile_pool(name="pss", bufs=3, space="PSUM") as pss, \
                 tc.tile_pool(name="pso", bufs=5, space="PSUM") as pso:
                for g in range(G):
                    for s in (3, 2, 1, 0):
                        q0 = s * 512
                        amax, nmax = plan[(g, s)]
                        outps = [pso.tile([128, 512], dt.float32, tag="outps",
                                          name="outps") for _ in range(4)]

                        def consume(kb, pt, off):
                            for j in range(4):
                                if 4 * s + j <= kb:
                                    nc.tensor.matmul(
                                        outps[j][:, :D + 1],
                                        pt[:, off + j * 128:off + (j + 1) * 128],
                                        VT[kb][:, g * (D + 1):(g + 1) * (D + 1)],
                                        start=(kb == amax), stop=(kb == 4 * s + j))

                        def width(kb):
                            return 512 if kb >= 4 * s + 3 else (kb - 4 * s + 1) * 128

                        pending = []
                        z2 = None
                        zoff = 0
                        zrec = []

                        def flush_pack():
                            nonlocal z2
                            pt2 = p4.tile([128, 1024], dt.bfloat16, tag="pt",
                                          bufs=4, name="pt2")
                            nc.scalar.activation(pt2[:, :zoff], z2[:, :zoff],
                                                 AF.Exp, scale=SCALE2)
                            for kbx, offx in zrec:
                                pending.append((kbx, pt2, offx))
                            z2 = None

                        for kb in range(amax, 4 * s - 1, -1):
                            w = width(kb)
                            sps = pss.tile([128, 512], dt.float32, tag="sps")
                            nc.tensor.matmul(sps[:, :w],
                                             KT[g][0][:, kb * 128:(kb + 1) * 128],
                                             QT[g][0][:, q0:q0 + w],
                                             start=True, stop=False)
                            nc.tensor.matmul(sps[:, :w],
                                             KT[g][1][:, kb * 128:(kb + 1) * 128],
                                             QT[g][1][:, q0:q0 + w],
                                             start=False, stop=True)
                            if len(pending) >= 4:
                                consume(*pending.pop(0))
                            if 4 * s + 4 <= kb <= nmax:
                                # fully inside the window: no mask needed
                                if z2 is not None:
                                    flush_pack()
                                pt1 = p4.tile([128, 512], dt.bfloat16, tag="pt1",
                                              bufs=4, name="pt1")
                                nc.scalar.activation(pt1[:, :w], sps[:, :w],
                                                     AF.Exp, scale=SCALE2)
                                pending.append((kb, pt1, 0))
                                continue
                            if z2 is None:
                                z2 = p4.tile([128, 1024], dt.bfloat16, tag="z",
                                             bufs=4, name="z2")
                                zoff, zrec = 0, []
                            nc.vector.scalar_tensor_tensor(
                                z2[:, zoff:zoff + w], wbig[:, q0:q0 + w],
                                kvec_t[:, g * 16 + kb:g * 16 + kb + 1], sps[:, :w],
                                op0=Alu.subtract, op1=Alu.min)
                            if kb <= 4 * s + 3:
                                nc.vector.tensor_tensor(
                                    out=z2[:, zoff + w - 128:zoff + w],
                                    in0=z2[:, zoff + w - 128:zoff + w],
                                    in1=dt_t[:], op=Alu.min)
                            zrec.append((kb, zoff))
                            zoff += w
                            if kb == 4 * s or zoff + width(kb - 1) > 1024:
                                flush_pack()
                        if z2 is not None:
                            flush_pack()
                        for it in pending:
                            consume(*it)
                        for j in (3, 2, 1, 0):
                            c = s * 4 + j
                            rden = p4.tile([128, 1], dt.float32, tag="rden",
                                           bufs=4, name="rden")
                            nc.vector.reciprocal(out=rden[:],
                                                 in_=outps[j][:, D:D + 1])
                            outn = p4.tile([128, D], dt.float32, tag="outn",
                                           bufs=4, name="outn")
                            if j % 2 == 0:
                                nc.scalar.mul(outn[:], outps[j][:, :D], rden[:])
                            else:
                                nc.vector.tensor_scalar(
                                    out=outn[:], in0=outps[j][:, :D],
                                    scalar1=rden[:], scalar2=None, op0=Alu.mult)
                            nc.sync.dma_start(
                                out=out[c * 128:(c + 1) * 128, g * D:(g + 1) * D],
                                in_=outn[:])

    nc.finalize()
    return nc


def _make_plan(query, woffl_np, lin2_b):
    """Host-exact window offsets -> per-(g,s) block plan (batch-uniform)."""
    z = np.maximum(query.astype(np.float64), 0.0).reshape(-1, H) @ woffl_np
    dx = (1.0 / (1.0 + np.exp(-(z + float(lin2_b[0]))))).reshape(B, L) * L
    plan = {}
    q_idx = np.arange(L, dtype=np.float64)
    for g, ws in enumerate(WS):
        lim = q_idx[None, :] + dx + ws          # [B, L] max allowed k (float)
        for s in range(4):
            sl = lim[:, s * 512:(s + 1) * 512]
            amax = 4 * s
            for kb in range(15, 4 * s - 1, -1):
                if not (kb * 128 > sl + MARGIN).all():
                    amax = kb
                    break
            nmax = 4 * s + 3
            for kb in range(min(amax, 15), 4 * s + 3, -1):
                if (kb * 128 + 127 <= sl - MARGIN).all():
                    nmax = kb
                    break
            plan[(g, s)] = (amax, nmax)
    return plan


def _prep_shared(qkv_w, qkv_b, off_w, lin2_w, lin2_b):
    f32 = np.float32
    bf = ml_dtypes.bfloat16
    qkv_wT = np.ascontiguousarray(qkv_w.T, dtype=f32)          # [H, 3H]
    woffl = (off_w.T.astype(np.float64) @ lin2_w.T.astype(np.float64))  # [H, 1]
    # [H, 2H] -> [p, hb, hin, c]; [H, H] -> [p, hin, c]  (partition-major)
    wqk_np = (qkv_wT[:, :2 * H].reshape(8, 128, 16, 128)
              .transpose(1, 2, 0, 3))
    wv_np = qkv_wT[:, 2 * H:].reshape(8, 128, H).transpose(1, 0, 2)
    shared = {
        "wqk": np.ascontiguousarray(wqk_np).astype(bf),
        "wv": np.ascontiguousarray(wv_np).astype(bf),
        "woffl": np.ascontiguousarray(woffl.reshape(8, 128).T).astype(bf),
        "bqk": np.ascontiguousarray(
            qkv_b[:2 * H].reshape(16, 128).T, dtype=f32),
        "bv": np.ascontiguousarray(
            np.broadcast_to(qkv_b[2 * H:][None], (128, H))).astype(bf),
        "blin2": np.asarray(lin2_b, dtype=f32).reshape(1, 1),
        "iotab": (BIG * np.arange(L, dtype=np.float64)).astype(f32)[None],
    }
    p = np.arange(128, dtype=np.float64)[:, None]
    cols = []
    for g in range(G):
        for kb in range(16):
            cols.append(BIG * (kb * 128 + p - WS[g]))
    shared["kvec"] = np.concatenate(cols, axis=1).astype(f32)
    pi = np.arange(128)[:, None]
    fi = np.arange(128)[None, :]
    shared["dtile"] = np.where(pi >= fi, 1e6, -1e6).astype(bf)
    return shared, woffl


def kernel(query, key_in, value, qkv_w, qkv_b, off_w, lin2_w, lin2_b,
           _trace=False, _tmpdir=None):
    query = np.asarray(query, dtype=np.float32)
    shared, woffl_np = _prep_shared(np.asarray(qkv_w, np.float32),
                                    np.asarray(qkv_b, np.float32),
                                    np.asarray(off_w, np.float32),
                                    np.asarray(lin2_w, np.float32),
                                    np.asarray(lin2_b, np.float32))
    plan = _make_plan(query, woffl_np, np.asarray(lin2_b, np.float64).ravel())
    in_maps = []
    for b in range(NCORES):
        m = dict(shared)
        # [p, strip, hin, col] so each 512-col strip is one contiguous dma run
        qT = (query[b].T.reshape(8, 128, 4, 512).transpose(1, 2, 0, 3))
        m["qtbf"] = np.ascontiguousarray(qT).astype(ml_dtypes.bfloat16)
        in_maps.append(m)

    key = tuple(sorted(plan.items()))
    if key not in _CACHE:
        _CACHE[key] = build_nc(plan)
    kw = {}
    if _trace:
        kw = dict(trace=True, tmpdir=_tmpdir)
    res = run_bass_kernel_spmd(_CACHE[key], in_maps,
                               core_ids=list(range(NCORES)), **kw)
    out = np.stack([np.asarray(res.results[b]["out"]) for b in range(NCORES)],
                   axis=0)
    if _trace:
        return out, res
    return out


if __name__ == "__main__":
    rng = np.random.default_rng(0)
    ins = {
        "query": rng.standard_normal((B, L, H)).astype(np.float32),
        "key_in": rng.standard_normal((B, L, H)).astype(np.float32),
        "value": rng.standard_normal((B, L, H)).astype(np.float32),
        "qkv_w": (rng.standard_normal((3 * H, H)) * 0.02).astype(np.float32),
        "qkv_b": np.zeros(3 * H, np.float32),
        "off_w": (rng.standard_normal((D1, H)) * 0.02).astype(np.float32),
        "lin2_w": (rng.standard_normal((1, D1)) * 0.02).astype(np.float32),
        "lin2_b": np.zeros(1, np.float32),
    }
    o = kernel(**ins)
    print("out", o.shape, o.dtype, np.abs(o).mean())
